# revision 1
# baseline (speedup 1.0000x reference)
"""Trainium2 Bass kernel for nn_MultiHeadAttention (B=2, S=2048, D=2048, H=16).

Sharding: tensor-parallel over heads -- each of the 8 cores owns 2 heads
(both batches) for the q/k/v projections and attention, then two 8-way
AllToAlls (one per local head, so the first overlaps the second head's
attention) convert the head-sharded attention output Y^T into a
token-sharded layout, and each core computes a disjoint 512-token slice of
the output projection (no all-reduce needed).

Layout trick: all projections are computed with the contraction dim on SBUF
partitions, producing Q^T/K^T in [dh, t] layout directly (scores are computed
transposed: S^T[j,i] = sum_dh K^T[dh,j] Q^T[dh,i]) so no on-device transposes
are ever needed.  Softmax over keys j (the partition dim of S^T) is done
without max-subtraction (scores are O(1) here) via exp on ACT; the row sums
are an all-ones matmul on the tensor engine which lands pre-broadcast across
partitions; 1/sum via DVE reciprocal_approx_fast.  All matmuls run as
float32r (full PE rate at N>=256); every tile feeding a matmul is written
with f32r output dtype to satisfy the walrus rounding rule.

Epilogues (psum -> sbuf + bias) run on the vector engine: ScalarE event
semaphores cost ~1.7-4us each on TRN2 and serialize ACT-paced tails.

Host does only data marshalling: transposes (x^T, w^T slices), sharding, and
the final concat/transpose of per-core output slices.
"""

import os
import sys

import numpy as np

_REPO = "/opt/trn_rl_repo"
if _REPO not in sys.path:
    sys.path.insert(0, _REPO)

from concourse import bacc, mybir, tile  # noqa: E402
import concourse.bass as bass  # noqa: E402

B, S, D, H = 2, 2048, 2048, 16
DH = D // H  # 128
NCORES = 8
HPC = H // NCORES  # heads per core = 2
JW = HPC * DH  # per-core head-feature width = 256
T = B * S  # 4096 flattened tokens
TSL = T // NCORES  # per-core output token slice = 512
SCALE = float(np.sqrt(DH))

F32 = mybir.dt.float32
F32R = mybir.dt.float32r
BF16 = mybir.dt.bfloat16
AF = mybir.ActivationFunctionType
ALU = bass.mybir.AluOpType

P = 128
IT = 512  # query i-tile width
NIT = S // IT  # 4 i-tiles per (batch, head)
NJC = S // P  # 16 key chunks per batch
NDC = D // P  # 16 contraction chunks
XSUB = 4  # x dc-chunks per sub-tile (finer DMA granularity)


def build_program():
    nc = bacc.Bacc(
        "TRN2",
        target_bir_lowering=False,
        debug=False,
        num_devices=NCORES,
    )

    # ---- kernel I/O (per-core values supplied via in_maps) ----
    xT = nc.dram_tensor("xT", [D, T], F32, kind="ExternalInput").ap()
    wqT = nc.dram_tensor("wqT", [D, JW], F32, kind="ExternalInput").ap()
    wkT = nc.dram_tensor("wkT", [D, JW], F32, kind="ExternalInput").ap()
    wvT = nc.dram_tensor("wvT", [D, JW], F32, kind="ExternalInput").ap()
    woT = nc.dram_tensor("woT", [D, D], BF16, kind="ExternalInput").ap()
    bq = nc.dram_tensor("bq", [JW], F32, kind="ExternalInput").ap()
    bk = nc.dram_tensor("bk", [JW], F32, kind="ExternalInput").ap()
    bv = nc.dram_tensor("bv", [JW], F32, kind="ExternalInput").ap()
    bo = nc.dram_tensor("bo", [D], F32, kind="ExternalInput").ap()
    # 4 diagonal-band mask patterns (1.0 = attend), [m][jj][ii]
    maskp = nc.dram_tensor("maskp", [4, P, IT], F32, kind="ExternalInput").ap()
    ones = nc.dram_tensor("ones", [P, P], F32, kind="ExternalInput").ap()
    out = nc.dram_tensor("out", [D, TSL], F32, kind="ExternalOutput").ap()

    xT_r = xT.rearrange("(dc p) t -> p dc t", p=P)

    with tile.TileContext(nc) as tc:
        with (
            tc.tile_pool(name="dram", bufs=1, space="DRAM") as dram,
            tc.tile_pool(name="const", bufs=1) as cpool,
            # attention-phase SBUF pools open early so k/v/q prefetch can
            # overlap the projection tail (PSUM pools stay phase-scoped)
            tc.tile_pool(name="kv", bufs=1) as kvpool,
            tc.tile_pool(name="small", bufs=2) as small,
            tc.tile_pool(name="epool", bufs=2) as epool,
        ):
            # DRAM scratch, split per (head, batch) for fine-grained deps
            qT_t = {}
            kT_t = {}
            for lh in range(HPC):
                for b in range(B):
                    qT_t[(lh, b)] = dram.tile([DH, S], F32, name=f"qT_{lh}_{b}")
                    kT_t[(lh, b)] = dram.tile([DH, S], F32, name=f"kT_{lh}_{b}")
            v_t = {b: dram.tile([S, JW], F32, name=f"v_{b}") for b in range(B)}
            # per-local-head AllToAll buffers (blocks = dest core's i-slice)
            a2a_in = {
                lh: dram.tile([NCORES, DH, TSL], BF16, name=f"a2a_in_{lh}")
                for lh in range(HPC)
            }
            a2a_out = {
                lh: dram.tile([NCORES * DH, TSL], BF16, name=f"a2a_out_{lh}")
                for lh in range(HPC)
            }

            # ---------- phase 1: q/k/v projections (to DRAM scratch) ----------
            with (
                tc.tile_pool(name="wpool", bufs=1) as wpool,
                tc.tile_pool(name="xpool", bufs=1) as xpool,
                tc.tile_pool(name="stage", bufs=3) as stage,
                tc.tile_pool(name="psum_p", bufs=3, space="PSUM") as psum_p,
            ):
                wq_sb = wpool.tile([P, NDC, JW], F32R, tag="wq")
                nc.sync.dma_start(
                    wq_sb[:], wqT.rearrange("(dc p) j -> p dc j", p=P).bitcast(F32R)
                )
                wk_sb = wpool.tile([P, NDC, JW], F32R, tag="wk")
                nc.sync.dma_start(
                    wk_sb[:], wkT.rearrange("(dc p) j -> p dc j", p=P).bitcast(F32R)
                )
                wv_sb = wpool.tile([P, NDC, JW], F32R, tag="wv")
                nc.sync.dma_start(
                    wv_sb[:], wvT.rearrange("(dc p) j -> p dc j", p=P).bitcast(F32R)
                )
                bq_sb = cpool.tile([P, HPC], F32)
                nc.sync.dma_start(bq_sb[:], bq.rearrange("(h p) -> p h", p=P))
                bk_sb = cpool.tile([P, HPC], F32)
                nc.sync.dma_start(bk_sb[:], bk.rearrange("(h p) -> p h", p=P))

                NTS = T // IT  # 8 token slices (batch 0 first, then batch 1)
                NXS = NDC // XSUB  # 4 x sub-tiles per slice
                for ts in range(NTS):
                    b, lt0 = ts // NIT, (ts % NIT) * IT
                    xs = []
                    for g in range(NXS):
                        xg = xpool.tile(
                            [P, XSUB, IT], F32R, tag="x", bufs=2 * NXS, name=f"x_{ts}_{g}"
                        )
                        nc.sync.dma_start(
                            xg[:],
                            xT_r[
                                :, g * XSUB : (g + 1) * XSUB, ts * IT : (ts + 1) * IT
                            ].bitcast(F32R),
                        )
                        xs.append(xg)

                    def xchunk(dc):
                        return xs[dc // XSUB][:, dc % XSUB, :]

                    # Q^T and K^T: psum[j(dh of head h), t]; epilogue on DVE
                    for w_sb, b_sb, out_t in (
                        (wk_sb, bk_sb, kT_t),
                        (wq_sb, bq_sb, qT_t),
                    ):
                        for h in range(HPC):
                            ps = psum_p.tile([P, IT], F32, tag="qk", name=f"pqk{ts}{h}")
                            for dc in range(NDC):
                                nc.tensor.matmul(
                                    ps[:],
                                    lhsT=w_sb[:, dc, h * DH : (h + 1) * DH],
                                    rhs=xchunk(dc),
                                    start=(dc == 0),
                                    stop=(dc == NDC - 1),
                                )
                            st = stage.tile([P, IT], F32, tag="qkst", name=f"sqk{ts}{h}")
                            nc.vector.tensor_tensor(
                                st[:],
                                ps[:],
                                b_sb[:, h : h + 1].to_broadcast([P, IT]),
                                ALU.add,
                            )
                            nc.sync.dma_start(out_t[(h, b)][:, lt0 : lt0 + IT], st[:])
                    # V: psum[t-chunk, j] (natural layout; bias applied later)
                    for tc2 in range(IT // P):
                        ps = psum_p.tile([P, JW], F32, tag="v", name=f"pv{ts}{tc2}")
                        for dc in range(NDC):
                            nc.tensor.matmul(
                                ps[:],
                                lhsT=xchunk(dc)[:, tc2 * P : (tc2 + 1) * P],
                                rhs=wv_sb[:, dc, :],
                                start=(dc == 0),
                                stop=(dc == NDC - 1),
                            )
                        stv = stage.tile([P, JW], F32, tag="vst", name=f"sv{ts}{tc2}")
                        nc.vector.tensor_copy(stv[:], ps[:])
                        r0 = lt0 + tc2 * P
                        nc.sync.dma_start(v_t[b][r0 : r0 + P, :], stv[:])

                # constants needed later (emitted last so they don't delay start)
                mask_sb = cpool.tile([P, 4, IT], F32)
                nc.sync.dma_start(mask_sb[:], maskp.rearrange("m p i -> p m i"))
                ones_sb = cpool.tile([P, P], F32R)
                nc.sync.dma_start(ones_sb[:], ones.bitcast(F32R))
                bv_sb = cpool.tile([P, HPC], F32)
                nc.sync.dma_start(bv_sb[:], bv.rearrange("(h p) -> p h", p=P))
                bo_sb = cpool.tile([P, NDC], F32)
                nc.sync.dma_start(bo_sb[:], bo.rearrange("(e p) -> p e", p=P))

            # ---------- phase 2: attention (head-sharded, causal) ----------
            with tc.tile_pool(name="wo", bufs=1) as wopool:
                wo_sb = wopool.tile([P, NDC, D], BF16)
                nc.sync.dma_start(
                    wo_sb[:], woT.rearrange("(jc p) e -> p jc e", p=P)
                )

                with (
                    tc.tile_pool(name="psS", bufs=2, space="PSUM") as psS,
                    tc.tile_pool(name="psO", bufs=2, space="PSUM") as psO,
                    tc.tile_pool(name="psR", bufs=2, space="PSUM") as psR,
                ):
                    for lh in range(HPC):
                        for b in range(B):
                            kT_sb = kvpool.tile([P, S], F32R, tag="k", bufs=2)
                            nc.sync.dma_start(
                                kT_sb[:], kT_t[(lh, b)][:].bitcast(F32R)
                            )
                            v_sb = kvpool.tile([P, NJC, DH], F32R, tag="v", bufs=2)
                            nc.sync.dma_start(
                                v_sb[:],
                                v_t[b][:, lh * DH : (lh + 1) * DH]
                                .rearrange("(tc p) d -> p tc d", p=P)
                                .bitcast(F32R),
                            )
                            for it in range(NIT):
                                q_sb = small.tile([P, IT], F32R, tag="q")
                                nc.sync.dma_start(
                                    q_sb[:],
                                    qT_t[(lh, b)][:, it * IT : (it + 1) * IT].bitcast(
                                        F32R
                                    ),
                                )
                                njc = (it + 1) * (IT // P)
                                po = psO.tile([P, IT], F32, tag="o")
                                pr = psR.tile([P, IT], F32, tag="r")
                                for jg in range(njc // 2):
                                    ps2 = psS.tile([P, 2, IT], F32, tag="s")
                                    for k2 in range(2):
                                        jc = jg * 2 + k2
                                        nc.tensor.matmul(
                                            ps2[:, k2, :],
                                            lhsT=kT_sb[:, jc * P : (jc + 1) * P],
                                            rhs=q_sb[:],
                                            start=True,
                                            stop=True,
                                        )
                                    e_sb = epool.tile([P, 2, IT], F32R, tag="e")
                                    nc.scalar.activation(
                                        e_sb[:], ps2[:], AF.Exp, scale=1.0 / SCALE
                                    )
                                    for k2 in range(2):
                                        jc = jg * 2 + k2
                                        if jc >= (it * IT) // P:
                                            m = jc - (it * IT) // P
                                            nc.vector.tensor_tensor(
                                                e_sb[:, k2, :],
                                                e_sb[:, k2, :],
                                                mask_sb[:, m, :],
                                                ALU.mult,
                                            )
                                        nc.tensor.matmul(
                                            po[:],
                                            lhsT=v_sb[:, jc, :],
                                            rhs=e_sb[:, k2, :],
                                            start=(jc == 0),
                                            stop=(jc == njc - 1),
                                        )
                                        nc.tensor.matmul(
                                            pr[:],
                                            lhsT=ones_sb[:],
                                            rhs=e_sb[:, k2, :],
                                            start=(jc == 0),
                                            stop=(jc == njc - 1),
                                        )
                                rinv = small.tile([P, IT], F32, tag="rinv")
                                nc.vector.reciprocal_approx_fast(rinv[:], pr[:])
                                y_sb = small.tile([P, IT], BF16, tag="y")
                                nc.vector.tensor_tensor(
                                    y_sb[:], po[:], rinv[:], ALU.mult
                                )
                                nc.vector.tensor_tensor(
                                    y_sb[:],
                                    y_sb[:],
                                    bv_sb[:, lh : lh + 1].to_broadcast([P, IT]),
                                    ALU.add,
                                )
                                g = NIT * b + it  # destination core / a2a block
                                nc.sync.dma_start(a2a_in[lh][g, :, :], y_sb[:])
                        # all-to-all for this head's rows; the lh=0 one
                        # overlaps the lh=1 attention on the compute engines
                        nc.gpsimd.collective_compute(
                            "AllToAll",
                            ALU.bypass,
                            replica_groups=[list(range(NCORES))],
                            ins=[a2a_in[lh][:].opt()],
                            outs=[a2a_out[lh][:].opt()],
                        )

                # ---------- phase 3: output projection on own token slice ----
                # rhs rows from a2a_out[lh]: block s holds global j rows
                # 256*s + 128*lh .. +128, i.e. key chunk jc = 2s + lh.
                with (
                    tc.tile_pool(name="ya", bufs=1) as yapool,
                    tc.tile_pool(name="ostage", bufs=2) as ostage,
                    tc.tile_pool(name="psout", bufs=2, space="PSUM") as psout,
                ):
                    ya_sb = {}
                    for lh in range(HPC):
                        ya_sb[lh] = yapool.tile(
                            [P, NCORES, TSL], BF16, name=f"ya{lh}"
                        )
                        nc.sync.dma_start(
                            ya_sb[lh][:],
                            a2a_out[lh][:].rearrange("(s p) i -> p s i", p=P),
                        )
                    EG = 4  # e-chunks per psum group
                    for eg in range(NDC // EG):
                        ps = psout.tile([P, EG, TSL], F32, tag="out")
                        for sub in range(EG):
                            ec = eg * EG + sub
                            for lh in range(HPC):
                                for s in range(NCORES):
                                    jc = 2 * s + lh
                                    nc.tensor.matmul(
                                        ps[:, sub, :],
                                        lhsT=wo_sb[:, jc, ec * P : ec * P + P],
                                        rhs=ya_sb[lh][:, s, :],
                                        start=(lh == 0 and s == 0),
                                        stop=(lh == HPC - 1 and s == NCORES - 1),
                                    )
                        ost = ostage.tile([P, EG, TSL], F32, tag="ost")
                        nc.vector.tensor_tensor(
                            ost[:],
                            ps[:],
                            bo_sb[:, eg * EG : (eg + 1) * EG, None].to_broadcast(
                                [P, EG, TSL]
                            ),
                            ALU.add,
                        )
                        nc.sync.dma_start(
                            out[eg * EG * P : (eg + 1) * EG * P, :].rearrange(
                                "(e p) i -> p e i", p=P
                            ),
                            ost[:],
                        )

    nc.finalize()  # bacc compile: regalloc etc. -- required before execution
    return nc


_PROGRAM = None


def _get_program():
    global _PROGRAM
    if _PROGRAM is None:
        _PROGRAM = build_program()
    return _PROGRAM


def _host_prep(x, mask, wq, bq, wk, bk, wv, bv, wo, bo):
    """Build the 8 per-core input maps (host-side marshalling only)."""
    f = np.float32
    x2 = np.asarray(x, dtype=f).reshape(T, D)
    xT = np.ascontiguousarray(x2.T)  # [D, T]
    import ml_dtypes
    woT = np.ascontiguousarray(np.asarray(wo, dtype=f).T).astype(ml_dtypes.bfloat16)  # [D, D]
    bo_ = np.ascontiguousarray(np.asarray(bo, dtype=f))

    # diagonal-band mask patterns from the provided mask (True = masked out).
    mask_np = np.asarray(mask)
    maskp = np.empty((4, P, IT), dtype=f)
    for m in range(4):
        maskp[m] = (~mask_np[0:IT, m * P : (m + 1) * P]).T.astype(f)
    maskp = np.ascontiguousarray(maskp)

    wq_, wk_, wv_ = (np.asarray(w, dtype=f) for w in (wq, wk, wv))
    bq_, bk_, bv_ = (np.asarray(v_, dtype=f) for v_ in (bq, bk, bv))

    in_maps = []
    for c in range(NCORES):
        j0, j1 = c * JW, (c + 1) * JW
        in_maps.append(
            {
                "xT": xT,
                "wqT": np.ascontiguousarray(wq_[j0:j1, :].T),
                "wkT": np.ascontiguousarray(wk_[j0:j1, :].T),
                "wvT": np.ascontiguousarray(wv_[j0:j1, :].T),
                "woT": woT,
                "bq": np.ascontiguousarray(bq_[j0:j1]),
                "bk": np.ascontiguousarray(bk_[j0:j1]),
                "bv": np.ascontiguousarray(bv_[j0:j1]),
                "bo": bo_,
                "maskp": maskp,
                "ones": np.ones((P, P), dtype=f),
            }
        )
    return in_maps


LAST_RESULTS = None  # BassKernelResults of the most recent run (for test.py)


def kernel(x, mask, wq, bq, wk, bk, wv, bv, wo, bo):
    global LAST_RESULTS
    from concourse.bass_utils import run_bass_kernel_spmd

    nc = _get_program()
    in_maps = _host_prep(x, mask, wq, bq, wk, bk, wv, bv, wo, bo)
    trace = os.environ.get("KERNEL_TRACE", "") == "1"
    kwargs = {}
    if os.environ.get("KERNEL_TRACE_ALL", "") == "1":
        kwargs["trace_cores"] = list(range(NCORES))
        kwargs["stitch_traces"] = True
    res = run_bass_kernel_spmd(
        nc, in_maps, core_ids=list(range(NCORES)), trace=trace, **kwargs
    )
    LAST_RESULTS = res
    # assemble: per-core out is out^T slice [D, 512]; concat on token axis,
    # transpose back to [T, D], reshape to [B, S, D]
    outT = np.concatenate([res.results[c]["out"] for c in range(NCORES)], axis=1)
    return np.ascontiguousarray(outT.T).reshape(B, S, D).astype(np.float32)



# revision 4
# speedup vs baseline: 1.1094x; 1.1094x over previous
"""Trainium2 Bass kernel for nn_MultiHeadAttention (B=2, S=2048, D=2048, H=16).

Sharding: tensor-parallel over heads -- each of the 8 cores owns 2 heads
(both batches) for the q/k/v projections and attention, then two 8-way
AllToAlls (one per local head, so the first overlaps the second head's
attention) convert the head-sharded attention output Y^T into a
token-sharded layout, and each core computes a disjoint 512-token slice of
the output projection (no all-reduce needed).

v2 changes vs the DRAM-scratch baseline:
- All matmul operands are bf16 (same PE rate as f32r at >=256 rows, half the
  DMA bytes and SBUF footprint); psum accumulation stays f32.
- q^T / k^T / v live entirely in SBUF between the projection and attention
  phases -- projection epilogues (DVE bias-add) write straight into the
  persistent tiles, no DRAM round-trip.
- Host pre-blocks every DRAM input into the exact [partition][...] layout the
  SBUF tiles want, so each DMA is ~128 large contiguous descriptors.
- Attention is software-pipelined: scores for group g+1 are issued before
  the AV/rowsum matmuls of group g, so the PE never stalls on the scalar
  engine's exp and stays at max p-state.
- Projection V matmuls (256 rows) are interleaved 1:1 with Q/K matmuls
  (512 rows) so LDWEIGHTS always hides under the previous matmul.
- Output projection accumulates all lh=0 (even key-chunk) contributions
  first so they execute during the second AllToAll, then lh=1.
"""

import os
import sys

import numpy as np

_REPO = "/opt/trn_rl_repo"
if _REPO not in sys.path:
    sys.path.insert(0, _REPO)

from concourse import bacc, mybir, tile  # noqa: E402
import concourse.bass as bass  # noqa: E402

B, S, D, H = 2, 2048, 2048, 16
DH = D // H  # 128
NCORES = 8
HPC = H // NCORES  # heads per core = 2
JW = HPC * DH  # per-core head-feature width = 256
T = B * S  # 4096 flattened tokens
TSL = T // NCORES  # per-core output token slice = 512
SCALE = float(np.sqrt(DH))

F32 = mybir.dt.float32
BF16 = mybir.dt.bfloat16
AF = mybir.ActivationFunctionType
ALU = bass.mybir.AluOpType

P = 128
IT = 512  # query i-tile width
NIT = S // IT  # 4 i-tiles per (batch, head)
NJC = S // P  # 16 key chunks per batch
NDC = D // P  # 16 contraction chunks
NTS = T // IT  # 8 token slices (batch 0 first, then batch 1)
XSUB = 4  # x dc-chunks per sub-tile (finer DMA granularity)
NXS = NDC // XSUB  # 4 x sub-tiles per slice


def build_program():
    nc = bacc.Bacc(
        "TRN2",
        target_bir_lowering=False,
        debug=False,
        num_devices=NCORES,
    )

    # ---- kernel I/O (host pre-blocked; per-core values via in_maps) ----
    xb = nc.dram_tensor("xb", [NTS, P, NDC, IT], BF16, kind="ExternalInput").ap()
    wqb = nc.dram_tensor("wqb", [P, NDC, JW], BF16, kind="ExternalInput").ap()
    wkb = nc.dram_tensor("wkb", [P, NDC, JW], BF16, kind="ExternalInput").ap()
    wvb = nc.dram_tensor("wvb", [P, NDC, JW], BF16, kind="ExternalInput").ap()
    # wo split by key-chunk parity: even chunks feed lh=0, odd feed lh=1
    woE = nc.dram_tensor("woE", [P, NJC // 2, D], BF16, kind="ExternalInput").ap()
    woO = nc.dram_tensor("woO", [P, NJC // 2, D], BF16, kind="ExternalInput").ap()
    bqb = nc.dram_tensor("bqb", [P, HPC], F32, kind="ExternalInput").ap()
    bkb = nc.dram_tensor("bkb", [P, HPC], F32, kind="ExternalInput").ap()
    bvb = nc.dram_tensor("bvb", [P, HPC], F32, kind="ExternalInput").ap()
    bob = nc.dram_tensor("bob", [P, NDC], F32, kind="ExternalInput").ap()
    # 4 diagonal-band mask patterns (1.0 = attend), [p][m][i]
    maskb = nc.dram_tensor("maskb", [P, 4, IT], BF16, kind="ExternalInput").ap()
    onesb = nc.dram_tensor("onesb", [P, P], BF16, kind="ExternalInput").ap()
    out = nc.dram_tensor("out", [P, NDC, TSL], F32, kind="ExternalOutput").ap()

    with tile.TileContext(nc) as tc:
        with (
            tc.tile_pool(name="dram", bufs=1, space="DRAM") as dram,
            tc.tile_pool(name="const", bufs=1) as cpool,
            tc.tile_pool(name="persist", bufs=1) as ppool,
            tc.tile_pool(name="small", bufs=2) as small,
            tc.tile_pool(name="epool", bufs=2) as epool,
        ):
            # ---- persistent SBUF tiles ----
            qT_sb = {}
            kT_sb = {}
            for lh in range(HPC):
                for b in range(B):
                    qT_sb[(lh, b)] = ppool.tile([P, S], BF16, name=f"qT_{lh}_{b}")
                    kT_sb[(lh, b)] = ppool.tile([P, S], BF16, name=f"kT_{lh}_{b}")
            v_sb = {
                b: ppool.tile([P, NJC, JW], BF16, name=f"v_{b}") for b in range(B)
            }
            ya_sb = {
                lh: ppool.tile([P, NCORES, TSL], BF16, name=f"ya{lh}")
                for lh in range(HPC)
            }
            wo_sb = {
                0: ppool.tile([P, NJC // 2, D], BF16, name="woE"),
                1: ppool.tile([P, NJC // 2, D], BF16, name="woO"),
            }

            # per-local-head AllToAll buffers (blocks = dest core's i-slice)
            a2a_in = {
                lh: dram.tile([NCORES, DH, TSL], BF16, name=f"a2a_in_{lh}")
                for lh in range(HPC)
            }
            a2a_out = {
                lh: dram.tile([NCORES * DH, TSL], BF16, name=f"a2a_out_{lh}")
                for lh in range(HPC)
            }

            # ---- constants / weights ----
            wq_w = cpool.tile([P, NDC, JW], BF16)
            wk_w = cpool.tile([P, NDC, JW], BF16)
            wv_w = cpool.tile([P, NDC, JW], BF16)
            nc.sync.dma_start(wk_w[:], wkb)
            nc.sync.dma_start(wq_w[:], wqb)
            nc.sync.dma_start(wv_w[:], wvb)
            bq_sb = cpool.tile([P, HPC], F32)
            bk_sb = cpool.tile([P, HPC], F32)
            bv_sb = cpool.tile([P, HPC], F32)
            bo_sb = cpool.tile([P, NDC], F32)
            nc.sync.dma_start(bk_sb[:], bkb)
            nc.sync.dma_start(bq_sb[:], bqb)
            nc.sync.dma_start(bv_sb[:], bvb)
            nc.sync.dma_start(bo_sb[:], bob)
            mask_sb = cpool.tile([P, 4, IT], BF16)
            nc.sync.dma_start(mask_sb[:], maskb)
            ones_sb = cpool.tile([P, P], BF16)
            nc.sync.dma_start(ones_sb[:], onesb)

            # ---------- phase 1: q/k/v projections (SBUF-resident) ----------
            with (
                tc.tile_pool(name="xpool", bufs=1) as xpool,
                tc.tile_pool(name="psum_p", bufs=1, space="PSUM") as psum_p,
            ):
                for ts in range(NTS):
                    b, lt0 = ts // NIT, (ts % NIT) * IT
                    xs = []
                    for g in range(NXS):
                        xg = xpool.tile(
                            [P, XSUB, IT], BF16, tag="x", bufs=6, name=f"x_{ts}_{g}"
                        )
                        nc.sync.dma_start(
                            xg[:], xb[ts, :, g * XSUB : (g + 1) * XSUB, :]
                        )
                        xs.append(xg)
                    if ts == 1:
                        nc.sync.dma_start(wo_sb[0][:], woE)
                    if ts == 3:
                        nc.sync.dma_start(wo_sb[1][:], woO)

                    def xchunk(dc):
                        return xs[dc // XSUB][:, dc % XSUB, :]

                    # 8 accumulation chains (K h0/h1, Q h0/h1 -> [j, t] psum;
                    # V tc0..3 -> [t, j] psum), advanced together per dc so
                    # 256-row V matmuls hide their LDWEIGHTS under 512-row
                    # Q/K matmuls.
                    pqk = {}
                    for i, nm in enumerate(("k0", "k1", "q0", "q1")):
                        pqk[nm] = psum_p.tile(
                            [P, IT], F32, tag=f"qk{i}", name=f"p{nm}_{ts}"
                        )
                    pv = {
                        tc2: psum_p.tile(
                            [P, JW], F32, tag=f"v{tc2}", name=f"pv{ts}_{tc2}"
                        )
                        for tc2 in range(IT // P)
                    }
                    for dc in range(NDC):
                        st, sp = dc == 0, dc == NDC - 1
                        for h in range(HPC):
                            nc.tensor.matmul(
                                pqk[f"k{h}"][:],
                                lhsT=wk_w[:, dc, h * DH : (h + 1) * DH],
                                rhs=xchunk(dc),
                                start=st,
                                stop=sp,
                            )
                            nc.tensor.matmul(
                                pv[h][:],
                                lhsT=xchunk(dc)[:, h * P : (h + 1) * P],
                                rhs=wv_w[:, dc, :],
                                start=st,
                                stop=sp,
                            )
                            nc.tensor.matmul(
                                pqk[f"q{h}"][:],
                                lhsT=wq_w[:, dc, h * DH : (h + 1) * DH],
                                rhs=xchunk(dc),
                                start=st,
                                stop=sp,
                            )
                            nc.tensor.matmul(
                                pv[2 + h][:],
                                lhsT=xchunk(dc)[:, (2 + h) * P : (3 + h) * P],
                                rhs=wv_w[:, dc, :],
                                start=st,
                                stop=sp,
                            )
                    # epilogues on DVE: bias add, write bf16 into persistents
                    for h in range(HPC):
                        nc.vector.tensor_tensor(
                            kT_sb[(h, b)][:, lt0 : lt0 + IT],
                            pqk[f"k{h}"][:],
                            bk_sb[:, h : h + 1].to_broadcast([P, IT]),
                            ALU.add,
                        )
                        nc.vector.tensor_tensor(
                            qT_sb[(h, b)][:, lt0 : lt0 + IT],
                            pqk[f"q{h}"][:],
                            bq_sb[:, h : h + 1].to_broadcast([P, IT]),
                            ALU.add,
                        )
                    for tc2 in range(IT // P):
                        # v bias is deferred to the attention epilogue
                        # (softmax rows sum to 1, so  attn @ (v+b) = attn@v + b)
                        nc.vector.tensor_copy(
                            v_sb[b][:, lt0 // P + tc2, :], pv[tc2][:]
                        )

            # ---------- phase 2: attention (head-sharded, causal) ----------
            with (
                tc.tile_pool(name="psS", bufs=2, space="PSUM") as psS,
                tc.tile_pool(name="psO", bufs=2, space="PSUM") as psO,
                tc.tile_pool(name="psR", bufs=2, space="PSUM") as psR,
            ):
                for lh in range(HPC):
                    for b in range(B):
                        kT = kT_sb[(lh, b)]
                        for it in range(NIT):
                            q_ap = qT_sb[(lh, b)][:, it * IT : (it + 1) * IT]
                            njc = (it + 1) * (IT // P)
                            po = psO.tile([P, IT], F32, tag="o")
                            pr = psR.tile([P, IT], F32, tag="r")

                            def emit_avr(e_tile, jg):
                                for k2 in range(2):
                                    jc = jg * 2 + k2
                                    nc.tensor.matmul(
                                        po[:],
                                        lhsT=v_sb[b][:, jc, lh * DH : (lh + 1) * DH],
                                        rhs=e_tile[:, k2, :],
                                        start=(jc == 0),
                                        stop=(jc == njc - 1),
                                    )
                                    nc.tensor.matmul(
                                        pr[:],
                                        lhsT=ones_sb[:],
                                        rhs=e_tile[:, k2, :],
                                        start=(jc == 0),
                                        stop=(jc == njc - 1),
                                    )

                            prev = None
                            for jg in range(njc // 2):
                                ps2 = psS.tile([P, 2, IT], F32, tag="s")
                                for k2 in range(2):
                                    jc = jg * 2 + k2
                                    nc.tensor.matmul(
                                        ps2[:, k2, :],
                                        lhsT=kT[:, jc * P : (jc + 1) * P],
                                        rhs=q_ap,
                                        start=True,
                                        stop=True,
                                    )
                                e_sb = epool.tile([P, 2, IT], BF16, tag="e", bufs=3)
                                nc.scalar.activation(
                                    e_sb[:], ps2[:], AF.Exp, scale=1.0 / SCALE
                                )
                                for k2 in range(2):
                                    jc = jg * 2 + k2
                                    if jc >= (it * IT) // P:
                                        m = jc - (it * IT) // P
                                        nc.vector.tensor_tensor(
                                            e_sb[:, k2, :],
                                            e_sb[:, k2, :],
                                            mask_sb[:, m, :],
                                            ALU.mult,
                                        )
                                if prev is not None:
                                    emit_avr(*prev)
                                prev = (e_sb, jg)
                            emit_avr(*prev)

                            rinv = small.tile([P, IT], F32, tag="rinv")
                            nc.vector.reciprocal_approx_fast(rinv[:], pr[:])
                            y_sb = small.tile([P, IT], BF16, tag="y")
                            nc.vector.tensor_tensor(
                                y_sb[:], po[:], rinv[:], ALU.mult
                            )
                            nc.vector.tensor_tensor(
                                y_sb[:],
                                y_sb[:],
                                bv_sb[:, lh : lh + 1].to_broadcast([P, IT]),
                                ALU.add,
                            )
                            g = NIT * b + it  # destination core / a2a block
                            nc.sync.dma_start(a2a_in[lh][g, :, :], y_sb[:])
                    # all-to-all for this head's rows; the lh=0 one
                    # overlaps the lh=1 attention on the compute engines
                    nc.gpsimd.collective_compute(
                        "AllToAll",
                        ALU.bypass,
                        replica_groups=[list(range(NCORES))],
                        ins=[a2a_in[lh][:].opt()],
                        outs=[a2a_out[lh][:].opt()],
                    )
                    nc.sync.dma_start(
                        ya_sb[lh][:],
                        a2a_out[lh][:].rearrange("(s p) i -> p s i", p=P),
                    )

            # ---------- phase 3: output projection on own token slice ----
            # ya_sb[lh] block s holds key chunk jc = 2s + lh, i.e. the s-th
            # chunk of wo_sb[lh] (parity-split).  All lh=0 matmuls for an
            # eg-pair are emitted first so they run during the lh=1 AllToAll.
            with (
                tc.tile_pool(name="ostage", bufs=2) as ostage,
                tc.tile_pool(name="psout", bufs=2, space="PSUM") as psout,
            ):
                EG = 4  # e-chunks per psum group
                NEG = NDC // EG
                for egp in range(NEG // 2):
                    pss = [
                        psout.tile([P, EG, TSL], F32, tag="out", name=f"po{egp}_{i}")
                        for i in range(2)
                    ]
                    for lh in range(HPC):
                        for half, ps in enumerate(pss):
                            eg = egp * 2 + half
                            for sub in range(EG):
                                ec = eg * EG + sub
                                for s in range(NCORES):
                                    nc.tensor.matmul(
                                        ps[:, sub, :],
                                        lhsT=wo_sb[lh][:, s, ec * P : ec * P + P],
                                        rhs=ya_sb[lh][:, s, :],
                                        start=(lh == 0 and s == 0),
                                        stop=(lh == HPC - 1 and s == NCORES - 1),
                                    )
                    for half, ps in enumerate(pss):
                        eg = egp * 2 + half
                        ost = ostage.tile([P, EG, TSL], F32, tag="ost")
                        nc.vector.tensor_tensor(
                            ost[:],
                            ps[:],
                            bo_sb[:, eg * EG : (eg + 1) * EG, None].to_broadcast(
                                [P, EG, TSL]
                            ),
                            ALU.add,
                        )
                        nc.sync.dma_start(out[:, eg * EG : (eg + 1) * EG, :], ost[:])

    nc.finalize()  # bacc compile: regalloc etc. -- required before execution
    return nc


_PROGRAM = None


def _get_program():
    global _PROGRAM
    if _PROGRAM is None:
        _PROGRAM = build_program()
    return _PROGRAM


def _host_prep(x, mask, wq, bq, wk, bk, wv, bv, wo, bo):
    """Build the 8 per-core input maps (host-side marshalling only)."""
    import ml_dtypes

    f = np.float32
    bf = ml_dtypes.bfloat16
    x2 = np.asarray(x, dtype=f).reshape(T, D)
    # [ts][p][dc][t] blocked x^T so every DMA descriptor is 4KB contiguous
    xb = x2.T.reshape(NDC, P, NTS, IT).transpose(2, 1, 0, 3).astype(bf)

    woT = np.asarray(wo, dtype=f).T.reshape(NJC, P, D)  # [jc][p][e]
    woE = woT[0::2].transpose(1, 0, 2).astype(bf)  # [p][s][e], jc = 2s
    woO = woT[1::2].transpose(1, 0, 2).astype(bf)  # [p][s][e], jc = 2s+1
    bo_b = np.ascontiguousarray(np.asarray(bo, dtype=f).reshape(NDC, P).T)

    # diagonal-band mask patterns from the provided mask (True = masked out)
    mask_np = np.asarray(mask)
    maskp = np.empty((4, P, IT), dtype=f)
    for m in range(4):
        maskp[m] = (~mask_np[0:IT, m * P : (m + 1) * P]).T.astype(f)
    maskb = maskp.transpose(1, 0, 2).astype(bf)  # [p][m][i]

    wq_, wk_, wv_ = (np.asarray(w, dtype=f) for w in (wq, wk, wv))
    bq_, bk_, bv_ = (np.asarray(v_, dtype=f) for v_ in (bq, bk, bv))

    in_maps = []
    for c in range(NCORES):
        j0, j1 = c * JW, (c + 1) * JW
        in_maps.append(
            {
                "xb": xb,
                "wqb": wq_[j0:j1, :].T.reshape(NDC, P, JW).transpose(1, 0, 2).astype(bf),
                "wkb": wk_[j0:j1, :].T.reshape(NDC, P, JW).transpose(1, 0, 2).astype(bf),
                "wvb": wv_[j0:j1, :].T.reshape(NDC, P, JW).transpose(1, 0, 2).astype(bf),
                "woE": woE,
                "woO": woO,
                "bqb": np.ascontiguousarray(bq_[j0:j1].reshape(HPC, P).T),
                "bkb": np.ascontiguousarray(bk_[j0:j1].reshape(HPC, P).T),
                "bvb": np.ascontiguousarray(bv_[j0:j1].reshape(HPC, P).T),
                "bob": bo_b,
                "maskb": maskb,
                "onesb": np.ones((P, P), dtype=bf),
            }
        )
    return in_maps


LAST_RESULTS = None  # BassKernelResults of the most recent run (for test.py)


def _assemble(per_core_outs):
    """[P, NDC, TSL] blocked slices -> full [B, S, D] output."""
    outT = np.concatenate(
        [
            np.asarray(o, dtype=np.float32).transpose(1, 0, 2).reshape(D, TSL)
            for o in per_core_outs
        ],
        axis=1,
    )
    return np.ascontiguousarray(outT.T).reshape(B, S, D).astype(np.float32)


def kernel(x, mask, wq, bq, wk, bk, wv, bv, wo, bo):
    global LAST_RESULTS
    from concourse.bass_utils import run_bass_kernel_spmd

    nc = _get_program()
    in_maps = _host_prep(x, mask, wq, bq, wk, bk, wv, bv, wo, bo)
    trace = os.environ.get("KERNEL_TRACE", "") == "1"
    kwargs = {}
    if os.environ.get("KERNEL_TRACE_ALL", "") == "1":
        kwargs["trace_cores"] = list(range(NCORES))
        kwargs["stitch_traces"] = True
    res = run_bass_kernel_spmd(
        nc, in_maps, core_ids=list(range(NCORES)), trace=trace, **kwargs
    )
    LAST_RESULTS = res
    return _assemble([res.results[c]["out"] for c in range(NCORES)])
